# revision 12
# baseline (speedup 1.0000x reference)
"""Trainium2 Bass kernel for nn_AttentionEdgeDecoder.

Reference computation (per batch b):
  hn = h[b,:4096,:], hg = h[b,4096,:]
  q = hg @ W_q  (single query, 8 heads x 16 dims)
  k,v = hn @ W_kv ; attn = softmax(q.k/sqrt(16)) ; y = attn.v
  mh = y @ W_mhc ; y2[i] = <mh, hn[i]>             (4096 scalars)
  e[i,j] = y2[j]*W_lin[0,0] + y2[i]*W_lin[1,0]     (4096x4096 output)

The output is 4*4096^2*4B = 268MB -> HBM-write bound. Sharding: 8 cores =
4 batches x 2 row-halves; each core computes y2[b] redundantly (tiny) and
streams its (2048, 4096) block of e to DRAM.

Math is reformulated transpose-free for the TensorEngine (out = lhsT.T @ rhs):
  q_col   = matmul(lhsT=W_q, rhs=hg_col)                 [128,1]
  Qh      = block-diag scatter of q_col                  [128,8]
  Wqeff   = matmul(lhsT=Wk^T, rhs=Qh) = Wk @ Qh          [128,8]
  sT      = matmul(lhsT=hnT_chunk, rhs=Wqeff) = hn@Wqeff [4096,8] (32 chunks)
  pT      = exp(0.25*sT)            (no max-subtract: |scores/4| < ~8)
  u_raw   = sum_chunks matmul(lhsT=pT_chunk, rhs=hn_chunk)  [8,128]
  ssum    = matmul(lhsT=ones, rhs=pT) -> reduce           [1,8]
  u       = u_raw * (1/ssum)                              [8,128]
  ymatT   = matmul(lhsT=Wv, rhs=u^T)                      [128,8]
  y_col   = per-head diagonal blocks of ymatT             [128,1]
  mh_row  = matmul(lhsT=y_col, rhs=W_mhc)                 [1,128]
  mh01    = matmul(lhsT=mh_row, rhs=W_lin_row)            [128,2] (outer prod)
  R0row   = matmul(lhsT=mh01[:,0], rhs=hnT) = W0*y2[j]    [1,4096]
  R       = matmul(lhsT=ones_row, rhs=R0row)  (partition broadcast) [128,4096]
  col     = matmul(lhsT=hrT_tile, rhs=mh01[:,1]) = W1*y2[rows]     [128,16]
  e_tile  = tensor_scalar_add(R, col[:,t])  -> DMA out
"""

from contextlib import ExitStack

import numpy as np

import concourse.bass as bass
import concourse.mybir as mybir
from concourse import bacc, tile
from concourse.bass_utils import run_bass_kernel_spmd
from concourse.masks import make_identity

BP = 4
N = 4096
HID = 128
H = 8
D = 16
ROWS = N // 2          # 2048 rows per core
NT = ROWS // 128       # 16 row tiles per core
NJC = N // 128         # 32 node chunks
F32 = mybir.dt.float32

# output chunking: OT row-tiles per DMA (4MB chunks)
TPC = 2                # row-tiles per output chunk
NCHUNK = NT // TPC     # 8 output chunks


def build_bass():
    nc = bacc.Bacc()

    hnT_ext = nc.declare_dram_parameter("hnT", [HID, N], F32, isOutput=False)
    hn_ext = nc.declare_dram_parameter("hn", [N, HID], F32, isOutput=False)
    hg_ext = nc.declare_dram_parameter("hg", [HID, 1], F32, isOutput=False)
    hrT_ext = nc.declare_dram_parameter("hrT", [HID, ROWS], F32, isOutput=False)
    wq_ext = nc.declare_dram_parameter("W_q", [HID, HID], F32, isOutput=False)
    wkv_ext = nc.declare_dram_parameter("W_kv", [HID, 2 * HID], F32, isOutput=False)
    wmhc_ext = nc.declare_dram_parameter("W_mhc", [HID, HID], F32, isOutput=False)
    wlin_ext = nc.declare_dram_parameter("W_lin_row", [1, 2], F32, isOutput=False)
    out_ext = nc.declare_dram_parameter("out", [ROWS, N], F32, isOutput=True)

    with tile.TileContext(nc) as tc, ExitStack() as ctx:
        consts = ctx.enter_context(tc.tile_pool(name="consts", bufs=1))
        sb = ctx.enter_context(tc.tile_pool(name="sb", bufs=1))
        small = ctx.enter_context(tc.tile_pool(name="small", bufs=1))
        ps_acc = ctx.enter_context(tc.tile_pool(name="ps_acc", bufs=1, space="PSUM"))
        ps_tmp = ctx.enter_context(tc.tile_pool(name="ps_tmp", bufs=2, space="PSUM"))
        epool = ctx.enter_context(tc.tile_pool(name="epool", bufs=3))

        # ---- constants ----
        identity = consts.tile([128, 128], F32)
        make_identity(nc, identity)
        ones_col = consts.tile([128, 1], F32)
        nc.vector.memset(ones_col[:], 1.0)
        ones_row = consts.tile([1, 128], F32)
        nc.vector.memset(ones_row[:], 1.0)
        # block-diagonal head mask: mask[p, h] = 1 iff p // 16 == h
        mask_sb = consts.tile([128, H], F32)
        nc.gpsimd.memset(mask_sb[:], 1.0)
        nc.gpsimd.affine_select(
            out=mask_sb[:], in_=mask_sb[:],
            compare_op=mybir.AluOpType.is_ge, fill=0.0,
            base=0, channel_multiplier=1, pattern=[[-D, H]],
        )
        nc.gpsimd.affine_select(
            out=mask_sb[:], in_=mask_sb[:],
            compare_op=mybir.AluOpType.is_ge, fill=0.0,
            base=D - 1, channel_multiplier=-1, pattern=[[D, H]],
        )

        # ---- input DMAs ----
        wq_sb = sb.tile([HID, HID], F32)
        nc.sync.dma_start(wq_sb[:], wq_ext[:, :])
        wkv_sb = sb.tile([HID, 2 * HID], F32)
        nc.sync.dma_start(wkv_sb[:], wkv_ext[:, :])
        wmhc_sb = sb.tile([HID, HID], F32)
        nc.sync.dma_start(wmhc_sb[:], wmhc_ext[:, :])
        wlin_sb = small.tile([1, 2], F32)
        nc.sync.dma_start(wlin_sb[:], wlin_ext[:, :])
        hg_sb = small.tile([HID, 1], F32)
        nc.sync.dma_start(hg_sb[:], hg_ext[:, :])

        hnT_sb = sb.tile([HID, N], F32)
        for k in range(4):
            nc.sync.dma_start(
                hnT_sb[:, bass.ts(k, N // 4)], hnT_ext[:, bass.ts(k, N // 4)]
            )
        # hn in [j-within-chunk, chunk, c] layout
        hn_sb = sb.tile([128, NJC, HID], F32)
        hn_r = hn_ext[:, :].rearrange("(jc p) c -> p jc c", p=128)
        for k in range(4):
            nc.sync.dma_start(
                hn_sb[:, bass.ts(k, NJC // 4), :], hn_r[:, bass.ts(k, NJC // 4), :]
            )
        hrT_sb = sb.tile([HID, ROWS], F32)
        for k in range(2):
            nc.sync.dma_start(
                hrT_sb[:, bass.ts(k, ROWS // 2)], hrT_ext[:, bass.ts(k, ROWS // 2)]
            )

        # ---- attention prologue ----
        # q_col = W_q.T @ hg
        q_ps = ps_tmp.tile([HID, 1], F32, tag="tmp")
        nc.tensor.matmul(q_ps[:], wq_sb[:], hg_sb[:], start=True, stop=True)
        q_sb = small.tile([HID, 1], F32)
        nc.scalar.copy(q_sb[:], q_ps[:])

        # Qh block-diag scatter: Qh[e, h] = mask[e, h] * q[e]
        qh_sb = small.tile([HID, H], F32)
        nc.vector.tensor_scalar_mul(qh_sb[:], mask_sb[:], q_sb[:])

        # WkT (transpose of W_kv[:, :HID])
        wkT_ps = ps_tmp.tile([HID, HID], F32, tag="tmp")
        nc.tensor.transpose(wkT_ps[:], wkv_sb[:, 0:HID], identity[:])
        wkT_sb = small.tile([HID, HID], F32)
        nc.scalar.copy(wkT_sb[:], wkT_ps[:])

        # Wqeff = Wk @ Qh
        wqeff_ps = ps_tmp.tile([HID, H], F32, tag="tmp")
        nc.tensor.matmul(wqeff_ps[:], wkT_sb[:], qh_sb[:], start=True, stop=True)
        wqeff_sb = small.tile([HID, H], F32)
        nc.scalar.copy(wqeff_sb[:], wqeff_ps[:])

        # sT chunks: [j, h] scores, packed into one PSUM tensor [128, 32, 8]
        sT_ps = ps_acc.tile([128, NJC, H], F32)
        for jc in range(NJC):
            nc.tensor.matmul(
                sT_ps[:, jc, :],
                hnT_sb[:, bass.ts(jc, 128)],
                wqeff_sb[:],
                start=True,
                stop=True,
            )
        # pT = exp(0.25 * sT)   (one ACT op over all 256 columns)
        pT_sb = small.tile([128, NJC, H], F32)
        nc.scalar.activation(
            pT_sb[:], sT_ps[:], mybir.ActivationFunctionType.Exp, scale=0.25
        )

        # u_raw[h, c] accumulation over 32 chunks
        u_ps = ps_acc.tile([H, HID], F32)
        for jc in range(NJC):
            nc.tensor.matmul(
                u_ps[:],
                pT_sb[:, jc, :],
                hn_sb[:, jc, :],
                start=(jc == 0),
                stop=(jc == NJC - 1),
            )

        # ssum[h] = sum_j pT[j, h] via ones-matmul, then reduce over chunks
        sums_ps = ps_tmp.tile([1, NJC * H], F32, tag="tmp")
        nc.tensor.matmul(
            sums_ps[:],
            ones_col[:],
            pT_sb[:].rearrange("p a b -> p (a b)"),
            start=True,
            stop=True,
        )
        ssum_sb = small.tile([1, H], F32)
        nc.vector.tensor_reduce(
            ssum_sb[:],
            sums_ps[:].rearrange("p (a b) -> p b a", b=H),
            axis=mybir.AxisListType.X,
            op=mybir.AluOpType.add,
        )
        rr_sb = small.tile([1, H], F32)
        nc.vector.reciprocal(rr_sb[:], ssum_sb[:])
        rr_ps = ps_tmp.tile([H, 1], F32, tag="tmp")
        nc.tensor.transpose(rr_ps[:], rr_sb[:], identity[0:1, 0:1])
        rs_sb = small.tile([H, 1], F32)
        nc.scalar.copy(rs_sb[:], rr_ps[:])

        # u = u_raw / ssum  (per-partition scalar multiply)
        u_sb = small.tile([H, HID], F32)
        nc.vector.tensor_scalar_mul(u_sb[:], u_ps[:], rs_sb[:])

        # uT
        uT_ps = ps_tmp.tile([HID, H], F32, tag="tmp")
        nc.tensor.transpose(uT_ps[:], u_sb[:], identity[0:H, 0:H])
        uT_sb = small.tile([HID, H], F32)
        nc.scalar.copy(uT_sb[:], uT_ps[:])

        # ymatT = Wv.T @ uT  -> [e, h]
        ymatT_ps = ps_tmp.tile([HID, H], F32, tag="tmp")
        nc.tensor.matmul(
            ymatT_ps[:], wkv_sb[:, HID : 2 * HID], uT_sb[:], start=True, stop=True
        )
        # y_col[e] = ymatT[e, head(e)] = sum_h ymatT[e, h] * mask[e, h]
        ymm_sb = small.tile([HID, H], F32)
        y_sb = small.tile([HID, 1], F32)
        nc.vector.tensor_mul(ymm_sb[:], ymatT_ps[:], mask_sb[:])
        nc.vector.tensor_reduce(
            y_sb[:], ymm_sb[:], axis=mybir.AxisListType.X, op=mybir.AluOpType.add
        )

        # mh_row = y.T @ W_mhc
        mh_ps = ps_tmp.tile([1, HID], F32, tag="tmp")
        nc.tensor.matmul(mh_ps[:], y_sb[:], wmhc_sb[:], start=True, stop=True)
        mh_sb = small.tile([1, HID], F32)
        nc.scalar.copy(mh_sb[:], mh_ps[:])

        # mh01[c, o] = mh[c] * W_lin[o]  (outer product, K=1)
        mh01_ps = ps_tmp.tile([HID, 2], F32, tag="tmp")
        nc.tensor.matmul(mh01_ps[:], mh_sb[:], wlin_sb[:], start=True, stop=True)
        mh01_sb = small.tile([HID, 2], F32)
        nc.scalar.copy(mh01_sb[:], mh01_ps[:])

        # R0row = (W0*mh).T @ hnT = W0*y2[j]  ; broadcast to 128 partitions
        r_sb = sb.tile([128, N], F32)
        r0_sb = small.tile([1, N], F32)
        for k in range(8):
            r0_ps = ps_tmp.tile([1, 512], F32, tag="big")
            nc.tensor.matmul(
                r0_ps[:],
                mh01_sb[:, 0:1],
                hnT_sb[:, bass.ts(k, 512)],
                start=True,
                stop=True,
            )
            nc.scalar.copy(r0_sb[:, bass.ts(k, 512)], r0_ps[:])
            rb_ps = ps_tmp.tile([128, 512], F32, tag="big")
            nc.tensor.matmul(
                rb_ps[:], ones_row[:], r0_sb[:, bass.ts(k, 512)], start=True, stop=True
            )
            if k % 2 == 0:
                nc.vector.tensor_copy(r_sb[:, bass.ts(k, 512)], rb_ps[:])
            else:
                nc.scalar.copy(r_sb[:, bass.ts(k, 512)], rb_ps[:])

        # col[p, t] = W1*y2[r0 + t*128 + p]
        col_ps = ps_acc.tile([128, NT], F32)
        for t in range(NT):
            nc.tensor.matmul(
                col_ps[:, t : t + 1],
                hrT_sb[:, bass.ts(t, 128)],
                mh01_sb[:, 1:2],
                start=True,
                stop=True,
            )
        col_sb = small.tile([128, NT], F32)
        nc.vector.tensor_copy(col_sb[:], col_ps[:])

        # ---- epilogue: e tiles + DMA out ----
        out_r = out_ext[:, :].rearrange("(o s p) j -> o p s j", s=TPC, p=128)
        for ot in range(NCHUNK):
            etile = epool.tile([128, TPC, N], F32)
            for s in range(TPC):
                t = ot * TPC + s
                nc.vector.tensor_scalar_add(
                    etile[:, s, :], r_sb[:], col_sb[:, t : t + 1]
                )
            nc.sync.dma_start(out_r[ot], etile[:])

    nc.finalize()
    return nc


_CACHED = {}


def _get_nc():
    if "nc" not in _CACHED:
        _CACHED["nc"] = build_bass()
    return _CACHED["nc"]


def kernel(h, W_q, W_kv, W_mhc, W_lin, _trace=False):
    h = np.ascontiguousarray(np.asarray(h, dtype=np.float32))
    W_q = np.ascontiguousarray(np.asarray(W_q, dtype=np.float32))
    W_kv = np.ascontiguousarray(np.asarray(W_kv, dtype=np.float32))
    W_mhc = np.ascontiguousarray(np.asarray(W_mhc, dtype=np.float32))
    W_lin = np.ascontiguousarray(np.asarray(W_lin, dtype=np.float32))

    nc = _get_nc()

    in_maps = []
    for core in range(8):
        b, half = core // 2, core % 2
        hn = h[b, :N, :]
        in_maps.append(
            {
                "hnT": np.ascontiguousarray(hn.T),
                "hn": hn,
                "hg": np.ascontiguousarray(h[b, N, :].reshape(HID, 1)),
                "hrT": np.ascontiguousarray(hn[half * ROWS : (half + 1) * ROWS, :].T),
                "W_q": W_q,
                "W_kv": W_kv,
                "W_mhc": W_mhc,
                "W_lin_row": np.ascontiguousarray(W_lin.reshape(1, 2)),
            }
        )

    import time as _time

    _t = _time.time()
    print("[kernel] launching run_bass_kernel_spmd", flush=True)
    kw = {}
    if _trace:
        import os

        kw = {"tmpdir": "/tmp/ktrace_" + str(os.getpid())}
        os.makedirs(kw["tmpdir"], exist_ok=True)
        print("[kernel] trace dir:", kw["tmpdir"], flush=True)
    res = run_bass_kernel_spmd(nc, in_maps, core_ids=list(range(8)), trace=_trace, **kw)
    print(f"[kernel] run_bass_kernel_spmd done in {_time.time()-_t:.1f}s", flush=True)

    out = np.empty((BP, N * N, 1), dtype=np.float32)
    for core in range(8):
        b, half = core // 2, core % 2
        blk = res.results[core]["out"]  # (2048, 4096)
        out[b, half * ROWS * N : (half + 1) * ROWS * N, 0] = blk.ravel()
    if _trace:
        return out, res
    return out


# revision 13
# speedup vs baseline: 1.2256x; 1.2256x over previous
"""Trainium2 Bass kernel for nn_AttentionEdgeDecoder.

Reference computation (per batch b):
  hn = h[b,:4096,:], hg = h[b,4096,:]
  q = hg @ W_q  (single query, 8 heads x 16 dims)
  k,v = hn @ W_kv ; attn = softmax(q.k/sqrt(16)) ; y = attn.v
  mh = y @ W_mhc ; y2[i] = <mh, hn[i]>             (4096 scalars)
  e[i,j] = y2[j]*W_lin[0,0] + y2[i]*W_lin[1,0]     (4096x4096 output)

Output is 4*4096^2*4B = 268MB -> HBM-write bound. Sharding: 8 cores =
4 batches x 2 row-halves; each core computes y2[b] redundantly (tiny) and
streams its (2048, 4096) block of e to DRAM at the per-core HBM limit.

TensorEngine formulation (out = lhsT.T @ rhs, all f32):
  q_col   = matmul(lhsT=W_q, rhs=hg_col)                  [128,1]
  Qh      = headmask * q_col   (block-diag scatter)       [128,8]
  Wqeff   = matmul(lhsT=Wk^T, rhs=Qh) = Wk @ Qh           [128,8]
  sT      = matmul(lhsT=hnT_chunk, rhs=Wqeff)             [4096,8] j-partitioned
  pT      = exp(0.25*sT)      (no max-subtract: |s/4| < 8)
  u_raw   = sum_chunks matmul(lhsT=pT_chunk, rhs=hn_chunk)   [8,128]
  ssum    = matmul(lhsT=ones, rhs=pT) -> strided reduce      [1,8]
  u       = u_raw * (1/ssum)                                 [8,128]
  ymatT   = matmul(lhsT=Wv, rhs=u^T)                         [128,8]
  y_col   = reduce_h(ymatT * headmask)                       [128,1]
  mh_row  = matmul(lhsT=y_col, rhs=W_mhc)                    [1,128]
  mh0_rep = (W0*mh_row) x ones  (outer, K=1)                 [128,128]
  R       = matmul(lhsT=mh0_rep, rhs=hnT) = bcast W0*y2[j]   [128,4096]
  mh1_col = matmul(lhsT=mh_row, rhs=W1)  (K=1)               [128,1]
  col     = matmul(lhsT=hrT_tile, rhs=mh1_col) = W1*y2[rows] [128,16]
  e_tile  = tensor_scalar_add(R, col[:,t])  -> 4MB DMAs out

A burst of dummy identity matmuls at t=0 warms the PE (HAM clock gate
4/8 -> 8/8) while input DMAs stream.
"""

from contextlib import ExitStack

import numpy as np

import concourse.bass as bass
import concourse.mybir as mybir
from concourse import bacc, tile
from concourse.bass_utils import run_bass_kernel_spmd
from concourse.masks import make_identity

BP = 4
N = 4096
HID = 128
H = 8
D = 16
ROWS = N // 2          # 2048 rows per core
NT = ROWS // 128       # 16 row tiles per core
NJC = N // 128         # 32 node chunks
F32 = mybir.dt.float32

TPC = 2                # row-tiles per output chunk (4MB DMAs)
NCHUNK = NT // TPC
NWARM = 28             # PE warm-up matmuls (~6us cold -> HAM 8/8)

# wpack column layout: [W_q | W_kv | W_mhc | hg | W_lin(row0)]
WQ0, WKV0, WMHC0, HG0, WL0 = 0, HID, 3 * HID, 4 * HID, 4 * HID + 1
WPACK_COLS = 4 * HID + 3


def build_bass():
    nc = bacc.Bacc()

    wpack_ext = nc.declare_dram_parameter("wpack", [HID, WPACK_COLS], F32, isOutput=False)
    hnT_ext = nc.declare_dram_parameter("hnT", [HID, N], F32, isOutput=False)
    hn_ext = nc.declare_dram_parameter("hn", [N, HID], F32, isOutput=False)
    hrT_ext = nc.declare_dram_parameter("hrT", [HID, ROWS], F32, isOutput=False)
    out_ext = nc.declare_dram_parameter("out", [ROWS, N], F32, isOutput=True)

    with tile.TileContext(nc) as tc, ExitStack() as ctx:
        consts = ctx.enter_context(tc.tile_pool(name="consts", bufs=1))
        sb = ctx.enter_context(tc.tile_pool(name="sb", bufs=1))
        small = ctx.enter_context(tc.tile_pool(name="small", bufs=1))
        ps_acc = ctx.enter_context(tc.tile_pool(name="ps_acc", bufs=1, space="PSUM"))
        ps_tmp = ctx.enter_context(tc.tile_pool(name="ps_tmp", bufs=2, space="PSUM"))
        epool = ctx.enter_context(tc.tile_pool(name="epool", bufs=3))

        # ---- constants ----
        identity = consts.tile([128, 128], F32)
        make_identity(nc, identity)
        ones_row = consts.tile([1, 128], F32)
        nc.vector.memset(ones_row[:], 1.0)
        ones_col = consts.tile([128, 1], F32)
        nc.vector.memset(ones_col[:], 1.0)
        # block-diagonal head mask: mask[p, h] = 1 iff p // 16 == h
        mask_sb = consts.tile([128, H], F32)
        nc.gpsimd.memset(mask_sb[:], 1.0)
        nc.gpsimd.affine_select(
            out=mask_sb[:], in_=mask_sb[:],
            compare_op=mybir.AluOpType.is_ge, fill=0.0,
            base=0, channel_multiplier=1, pattern=[[-D, H]],
        )
        nc.gpsimd.affine_select(
            out=mask_sb[:], in_=mask_sb[:],
            compare_op=mybir.AluOpType.is_ge, fill=0.0,
            base=D - 1, channel_multiplier=-1, pattern=[[D, H]],
        )

        # ---- PE warm-up: dense dependency-free matmuls from t=0 ----
        warm_ps = ps_acc.tile([128, 128], F32)
        for w in range(NWARM):
            nc.tensor.matmul(
                warm_ps[:], identity[:], identity[:], start=True, stop=True
            )

        # ---- input DMAs (sync/HWDGE ring, program order = issue order) ----
        wpack_sb = sb.tile([HID, WPACK_COLS], F32)
        nc.sync.dma_start(wpack_sb[:], wpack_ext[:, :])
        hnT_sb = sb.tile([HID, N], F32)
        for k in range(8):
            nc.sync.dma_start(
                hnT_sb[:, bass.ts(k, N // 8)], hnT_ext[:, bass.ts(k, N // 8)]
            )
        hn_sb = sb.tile([128, NJC, HID], F32)
        hn_r = hn_ext[:, :].rearrange("(jc p) c -> p jc c", p=128)
        for k in range(8):
            nc.sync.dma_start(
                hn_sb[:, bass.ts(k, NJC // 8), :], hn_r[:, bass.ts(k, NJC // 8), :]
            )
        hrT_sb = sb.tile([HID, ROWS], F32)
        for k in range(2):
            nc.sync.dma_start(
                hrT_sb[:, bass.ts(k, ROWS // 2)], hrT_ext[:, bass.ts(k, ROWS // 2)]
            )

        # ---- attention prologue ----
        # q_col = W_q.T @ hg
        q_ps = ps_tmp.tile([HID, 1], F32, tag="tmp")
        nc.tensor.matmul(
            q_ps[:], wpack_sb[:, WQ0:WQ0 + HID], wpack_sb[:, HG0:HG0 + 1],
            start=True, stop=True,
        )
        q_sb = small.tile([HID, 1], F32)
        nc.scalar.copy(q_sb[:], q_ps[:])

        # Qh block-diag scatter: Qh[e, h] = mask[e, h] * q[e]
        qh_sb = small.tile([HID, H], F32)
        nc.vector.tensor_scalar_mul(qh_sb[:], mask_sb[:], q_sb[:])

        # WkT (transpose of W_kv[:, :HID])
        wkT_ps = ps_tmp.tile([HID, HID], F32, tag="tmp")
        nc.tensor.transpose(wkT_ps[:], wpack_sb[:, WKV0:WKV0 + HID], identity[:])
        wkT_sb = small.tile([HID, HID], F32)
        nc.scalar.copy(wkT_sb[:], wkT_ps[:])

        # Wqeff = Wk @ Qh
        wqeff_ps = ps_tmp.tile([HID, H], F32, tag="tmp")
        nc.tensor.matmul(wqeff_ps[:], wkT_sb[:], qh_sb[:], start=True, stop=True)
        wqeff_sb = small.tile([HID, H], F32)
        nc.scalar.copy(wqeff_sb[:], wqeff_ps[:])

        # sT chunks: [j, h] scores, packed into one PSUM tensor [128, 32, 8]
        sT_ps = ps_acc.tile([128, NJC, H], F32)
        for jc in range(NJC):
            nc.tensor.matmul(
                sT_ps[:, jc, :],
                hnT_sb[:, bass.ts(jc, 128)],
                wqeff_sb[:],
                start=True,
                stop=True,
            )
        # pT = exp(0.25 * sT)   (one ACT op over all 256 columns)
        pT_sb = small.tile([128, NJC, H], F32)
        nc.scalar.activation(
            pT_sb[:], sT_ps[:], mybir.ActivationFunctionType.Exp, scale=0.25
        )

        # u_raw[h, c] accumulation over 32 chunks
        u_ps = ps_acc.tile([H, HID], F32)
        for jc in range(NJC):
            nc.tensor.matmul(
                u_ps[:],
                pT_sb[:, jc, :],
                hn_sb[:, jc, :],
                start=(jc == 0),
                stop=(jc == NJC - 1),
            )

        # ssum[h] = sum_j pT[j, h] via ones-matmul, then reduce over chunks
        sums_ps = ps_tmp.tile([1, NJC * H], F32, tag="tmp")
        nc.tensor.matmul(
            sums_ps[:],
            ones_col[:],
            pT_sb[:].rearrange("p a b -> p (a b)"),
            start=True,
            stop=True,
        )
        ssum_sb = small.tile([1, H], F32)
        nc.vector.tensor_reduce(
            ssum_sb[:],
            sums_ps[:].rearrange("p (a b) -> p b a", b=H),
            axis=mybir.AxisListType.X,
            op=mybir.AluOpType.add,
        )
        rr_sb = small.tile([1, H], F32)
        nc.vector.reciprocal(rr_sb[:], ssum_sb[:])
        rr_ps = ps_tmp.tile([H, 1], F32, tag="tmp")
        nc.tensor.transpose(rr_ps[:], rr_sb[:], identity[0:1, 0:1])
        rs_sb = small.tile([H, 1], F32)
        nc.scalar.copy(rs_sb[:], rr_ps[:])

        # u = u_raw / ssum  (per-partition scalar multiply)
        u_sb = small.tile([H, HID], F32)
        nc.vector.tensor_scalar_mul(u_sb[:], u_ps[:], rs_sb[:])

        # uT
        uT_ps = ps_tmp.tile([HID, H], F32, tag="tmp")
        nc.tensor.transpose(uT_ps[:], u_sb[:], identity[0:H, 0:H])
        uT_sb = small.tile([HID, H], F32)
        nc.scalar.copy(uT_sb[:], uT_ps[:])

        # ymatT = Wv.T @ uT  -> [e, h]
        ymatT_ps = ps_tmp.tile([HID, H], F32, tag="tmp")
        nc.tensor.matmul(
            ymatT_ps[:], wpack_sb[:, WKV0 + HID:WKV0 + 2 * HID], uT_sb[:],
            start=True, stop=True,
        )
        # y_col[e] = ymatT[e, head(e)] = sum_h ymatT[e, h] * mask[e, h]
        ymm_sb = small.tile([HID, H], F32)
        y_sb = small.tile([HID, 1], F32)
        nc.vector.tensor_mul(ymm_sb[:], ymatT_ps[:], mask_sb[:])
        nc.vector.tensor_reduce(
            y_sb[:], ymm_sb[:], axis=mybir.AxisListType.X, op=mybir.AluOpType.add
        )

        # mh_row = y.T @ W_mhc
        mh_ps = ps_tmp.tile([1, HID], F32, tag="tmp")
        nc.tensor.matmul(
            mh_ps[:], y_sb[:], wpack_sb[:, WMHC0:WMHC0 + HID], start=True, stop=True
        )
        mh_sb = small.tile([1, HID], F32)
        nc.scalar.copy(mh_sb[:], mh_ps[:])

        # mh0_row = W0 * mh_row ; mh0_rep[c, p] = mh0[c] (128 identical cols)
        mh0_sb = small.tile([1, HID], F32)
        nc.vector.tensor_scalar_mul(mh0_sb[:], mh_sb[:], wpack_sb[0:1, WL0:WL0 + 1])
        mh0rep_ps = ps_tmp.tile([HID, HID], F32, tag="tmp")
        nc.tensor.matmul(mh0rep_ps[:], mh0_sb[:], ones_row[:], start=True, stop=True)
        mh0rep_sb = small.tile([HID, HID], F32)
        nc.scalar.copy(mh0rep_sb[:], mh0rep_ps[:])

        # mh1_col[c] = mh[c] * W1  (K=1 outer product with scalar)
        mh1_ps = ps_tmp.tile([HID, 1], F32, tag="tmp")
        nc.tensor.matmul(
            mh1_ps[:], mh_sb[:], wpack_sb[0:1, WL0 + 1:WL0 + 2], start=True, stop=True
        )
        mh1_sb = small.tile([HID, 1], F32)
        nc.scalar.copy(mh1_sb[:], mh1_ps[:])

        # R[p, j] = W0*y2[j]: lhsT = mh0_rep (stationary), rhs = hnT chunks
        r_sb = sb.tile([128, N], F32)
        for k in range(8):
            rb_ps = ps_tmp.tile([128, 512], F32, tag="big")
            nc.tensor.matmul(
                rb_ps[:], mh0rep_sb[:], hnT_sb[:, bass.ts(k, 512)],
                start=True, stop=True,
            )
            if k % 2 == 0:
                nc.vector.tensor_copy(r_sb[:, bass.ts(k, 512)], rb_ps[:])
            else:
                nc.scalar.copy(r_sb[:, bass.ts(k, 512)], rb_ps[:])

        # col[p, t] = W1*y2[r0 + t*128 + p]
        col_ps = ps_acc.tile([128, NT], F32)
        for t in range(NT):
            nc.tensor.matmul(
                col_ps[:, t : t + 1],
                hrT_sb[:, bass.ts(t, 128)],
                mh1_sb[:],
                start=True,
                stop=True,
            )
        col_sb = small.tile([128, NT], F32)
        nc.vector.tensor_copy(col_sb[:], col_ps[:])

        # ---- epilogue: e tiles + DMA out ----
        out_r = out_ext[:, :].rearrange("(o s p) j -> o p s j", s=TPC, p=128)
        for ot in range(NCHUNK):
            etile = epool.tile([128, TPC, N], F32)
            for s in range(TPC):
                t = ot * TPC + s
                nc.vector.tensor_scalar_add(
                    etile[:, s, :], r_sb[:], col_sb[:, t : t + 1]
                )
            nc.sync.dma_start(out_r[ot], etile[:])

    nc.finalize()
    return nc


_CACHED = {}


def _get_nc():
    if "nc" not in _CACHED:
        _CACHED["nc"] = build_bass()
    return _CACHED["nc"]


def kernel(h, W_q, W_kv, W_mhc, W_lin, _trace=False):
    h = np.ascontiguousarray(np.asarray(h, dtype=np.float32))
    W_q = np.ascontiguousarray(np.asarray(W_q, dtype=np.float32))
    W_kv = np.ascontiguousarray(np.asarray(W_kv, dtype=np.float32))
    W_mhc = np.ascontiguousarray(np.asarray(W_mhc, dtype=np.float32))
    W_lin = np.ascontiguousarray(np.asarray(W_lin, dtype=np.float32))

    nc = _get_nc()

    wpack = np.zeros((HID, WPACK_COLS), dtype=np.float32)
    wpack[:, WQ0:WQ0 + HID] = W_q
    wpack[:, WKV0:WKV0 + 2 * HID] = W_kv
    wpack[:, WMHC0:WMHC0 + HID] = W_mhc
    wpack[0, WL0] = W_lin[0, 0]
    wpack[0, WL0 + 1] = W_lin[1, 0]

    in_maps = []
    for core in range(8):
        b, half = core // 2, core % 2
        hn = h[b, :N, :]
        wp = wpack.copy()
        wp[:, HG0] = h[b, N, :]
        in_maps.append(
            {
                "wpack": wp,
                "hnT": np.ascontiguousarray(hn.T),
                "hn": hn,
                "hrT": np.ascontiguousarray(hn[half * ROWS : (half + 1) * ROWS, :].T),
            }
        )

    import time as _time

    kw = {}
    if _trace:
        import os

        kw = {"tmpdir": "/tmp/ktrace_" + str(os.getpid())}
        os.makedirs(kw["tmpdir"], exist_ok=True)
        print("[kernel] trace dir:", kw["tmpdir"], flush=True)
    _t = _time.time()
    print("[kernel] launching run_bass_kernel_spmd", flush=True)
    res = run_bass_kernel_spmd(nc, in_maps, core_ids=list(range(8)), trace=_trace, **kw)
    print(f"[kernel] run_bass_kernel_spmd done in {_time.time()-_t:.1f}s", flush=True)

    out = np.empty((BP, N * N, 1), dtype=np.float32)
    for core in range(8):
        b, half = core // 2, core % 2
        blk = res.results[core]["out"]  # (2048, 4096)
        out[b, half * ROWS * N : (half + 1) * ROWS * N, 0] = blk.ravel()
    if _trace:
        return out, res
    return out
